# revision 19
# baseline (speedup 1.0000x reference)
"""Trainium2 Bass kernel for the 2-layer CIN — v5.

Math (per batch b, reference):
  x1[b,h,k] = sum_{i,j} W1[h,i,j] * x[b,i,k] * x[b,j,k] + b1[h]
  x2[b,h,k] = sum_{i,j} W2[h,i,j] * x1[b,i,k] * x[b,j,k] + b2[h]
  out[b, :] = [sum_k x1[b,:,k], sum_k x2[b,:,k]]          # [B, 256]

Device strategy (pure data parallel over 8 cores, 256 batches each):
  - Columns col=(b_lo 4, k 32) on the 128 SBUF partitions; 64 col-tiles.
  - Z rows: row 0 = 1.0 (bias carrier); rows 1+d*26+p = a_p * a_{(p+d)%26}
    (d in 0..13, 0.5-coeff fold for d=13); rows 365..383 zero.
  - T[dq, (j,bl)] = sum_k Z[(bl,k), dq] * x[b_bl, j, k] via per-(tile,
    chunk) matmuls contracting over the partition dim.  Canonical-triple
    fold: T[(p,q), j] is only consumed at j >= max(p,q), so chunk 1
    skips j<5 and chunk 2 skips j<10 (exact — weights re-bucketed on
    host into E).  Per-tile moving cols: 108 / 88 / 68.
  - out2[h,b] = sum_{dq,j} E[dq,j,h] * T[dq,(j,b)] — 63 accumulating
    matmuls (26+21+16) instead of 78.
  - out1[h,b] = sum_dq C[dq,h] * ZS[dq,b] — 3 accumulating matmuls on
    the ones-columns of T.
  - Z build on DVE; PSUM evacuation split across ACT / Pool / (order)
    with 2-group fused ops.  A/B probe: even groups build Z with
    per-tile scalar_tensor_tensor (4x_2p candidate), odd groups with
    per-group tensor_tensor (2x_1p).
  - Output stays [h, b] on device; the [b, h] transpose happens on host.
"""

import dataclasses
import os
import sys

sys.path.insert(0, "/opt/trn_rl_repo")

import numpy as np
import ml_dtypes

import concourse.bass as bass
import concourse.tile as tile
from concourse import bacc
from concourse import mybir
from concourse.bass_utils import run_bass_kernel_spmd

BF = ml_dtypes.bfloat16

B, M, K, H = 2048, 26, 32, 128
NC = 8
BS = B // NC        # 256 batches per core
NT = BS // 4        # 64 col tiles
NG = NT // 4        # 16 groups of 4 tiles
PQ = 384            # packed pair dim (3 chunks of 128)
AE = 48             # per-tile stride in a_ext

SKIP = (0, 5, 10)                 # first executed j per chunk
TC = tuple((26 - s) * 4 + 4 for s in SKIP)   # T cols/tile/chunk: 108, 88, 68
NJ = sum(26 - s for s in SKIP)    # out2 matmul count: 63

F32 = mybir.dt.float32
BF16 = mybir.dt.bfloat16
GS = 432             # per-group col stride in each tsb chunk region


def _sl(ap, ap_dims, extra_off=0):
    """Raw AP with custom free dims [(step, count), ...]."""
    return dataclasses.replace(
        ap, offset=ap.offset + extra_off,
        ap=[list(ap.ap[0])] + [[s, c] for s, c in ap_dims])


def build_nc():
    nc = bacc.Bacc("TRN2", target_bir_lowering=False, debug=False,
                   num_devices=NC)

    dr = lambda n, shp, dt: nc.dram_tensor(n, shp, dt, kind="ExternalInput").ap()
    aext_d = dr("aext", [128, NT * AE], BF16)
    as_d = dr("asd", [128, NT * 108], BF16)
    c_d = dr("c_w", [128, PQ], BF16)
    d_d = dr("d_w", [128, NJ * 128], BF16)
    b2_d = dr("b2s", [128, 1], F32)
    res_d = nc.dram_tensor("res", [128, 512], F32, kind="ExternalOutput").ap()

    with tile.TileContext(nc, trace_sim=False) as tc:
        _body(nc, aext_d, as_d, c_d, d_d, b2_d, res_d)
    nc.compile()
    return nc


def _body(nc, aext_d, as_d, c_d, d_d, b2_d, res_d):
    sb = lambda n, f, dt: nc.alloc_sbuf_tensor(n, [128, f], dt).ap()

    aext = sb("aext_s", NT * AE, BF16)
    asb = sb("asb", NT * 108, BF16)
    zbuf = sb("zbuf", NT * PQ, BF16)
    # tsb: 3 chunk regions of NG*GS cols each; group g's 4 tiles live at
    # [c*NG*GS + g*GS : +4*TC[c]] (tail of each 432-block junk for c1/c2).
    tsb = sb("tsb", 3 * NG * GS, BF16)
    csb = sb("csb", PQ, BF16)
    dsb = sb("dsb", NJ * 128, BF16)
    b2s = sb("b2s_s", 1, F32)
    ress = sb("ress", 512, F32)
    wsrc = sb("wsrc", 256, BF16)        # never written: warm-up junk

    # Per-chunk PSUM pools: c0/c1 depth-2, c2 depth-3; accp+o1p share the
    # 8th bank (accp = pacc[:, :256], o1p = pacc[:, 256:]).
    pc = [nc.alloc_psum_tensor("pc0", [128, 1024], F32).ap(),
          nc.alloc_psum_tensor("pc1", [128, 1024], F32).ap(),
          nc.alloc_psum_tensor("pc2", [128, 1536], F32).ap()]
    pacc = nc.alloc_psum_tensor("pacc", [128, 512], F32).ap()
    accp = pacc[:, 0:256]
    o1p = pacc[:, 256:512]
    DEP = (2, 2, 3)

    # ---- PE warm-up: no data deps, runs from preamble end ----
    for w in range(12):
        nc.tensor.matmul(o1p, wsrc[:, 0:128], wsrc[:, 0:256],
                         start=True, stop=True, skip_group_check=True)

    # ---- loads: ONE queue = strict FIFO drain order. A chunks first
    #      (aext/asd interleaved, progressive sizes), then csb/b2, then
    #      dsb (needed only by the mid-phase out2 half-A).
    for lo, n in ((0, 4), (4, 4), (8, 8), (16, 16), (32, 32)):
        s = slice(lo * AE, (lo + n) * AE)
        nc.sync.dma_start(aext[:, s], aext_d[:, s])
        s = slice(lo * 108, (lo + n) * 108)
        nc.sync.dma_start(asb[:, s], as_d[:, s])
    nc.sync.dma_start(csb, c_d)
    nc.sync.dma_start(b2s, b2_d)
    for lo, hi in ((0, 32), (32, NJ)):
        s = slice(lo * 128, hi * 128)
        nc.sync.dma_start(dsb[:, s], d_d[:, s])

    # ---- Z bias/zero rows (once, whole zbuf) ----
    nc.gpsimd.memset(_sl(zbuf, [(PQ, NT), (1, 1)], 0), 1.0)
    nc.gpsimd.memset(_sl(zbuf, [(PQ, NT), (1, 19)], 365), 0.0)

    def t_matmuls(g):
        for c in range(3):
            po = (g % DEP[c]) * 512
            for tt in range(4):
                t = g * 4 + tt
                nc.tensor.matmul(
                    pc[c][:, po + tt * TC[c]: po + (tt + 1) * TC[c]],
                    zbuf[:, t * PQ + c * 128: t * PQ + (c + 1) * 128],
                    asb[:, t * 108 + SKIP[c] * 4:(t + 1) * 108],
                    start=True, stop=True, skip_group_check=True)

    def evacs(g):
        # exact-size per-chunk ops; c2 on DVE except the early odd groups
        # (DVE is still streaming Z then); late c1s also to DVE to
        # balance the saturated ACT.
        for c in range(3):
            dst = tsb[:, c * NG * GS + g * GS:
                         c * NG * GS + g * GS + 4 * TC[c]]
            src = pc[c][:, (g % DEP[c]) * 512:
                          (g % DEP[c]) * 512 + 4 * TC[c]]
            dve = (c == 2 and not (g % 2 == 1 and g < 8)) or \
                  (c == 1 and g in (13, 15))
            if dve:
                nc.vector.tensor_scalar(dst, src, 0.0, None,
                                        mybir.AluOpType.add)
            else:
                nc.scalar.copy(dst, src)

    def half_ops(h):
        """Closure list: 3 out1 then 63 out2 accumulating matmuls for
        half h (out1 first so its store can overlap the out2 chain)."""
        ops = []
        for c in range(3):
            def mm1(c=c):
                nc.tensor.matmul(
                    o1p[:, h * 128:(h + 1) * 128],
                    csb[:, c * 128:(c + 1) * 128],
                    _sl(tsb, [(GS, 8), (TC[c], 4), (1, 4)],
                        c * NG * GS + h * 8 * GS + TC[c] - 4),
                    start=(c == 0), stop=(c == 2), skip_group_check=True)
            ops.append(mm1)
        blk = 0
        for c in range(3):
            for j in range(SKIP[c], 26):
                def mm(c=c, j=j, blk=blk):
                    nc.tensor.matmul(
                        accp[:, h * 128:h * 128 + 128],
                        dsb[:, blk * 128:(blk + 1) * 128],
                        _sl(tsb, [(GS, 8), (TC[c], 4), (1, 4)],
                            c * NG * GS + h * 8 * GS + (j - SKIP[c]) * 4),
                        start=(blk == 0), stop=(blk == NJ - 1),
                        skip_group_check=True)
                ops.append(mm)
                blk += 1
        return ops

    # ---- T phase; half-A of out2/out1 issue-interleaved into the PE
    #      slack of groups 9..15 (its deps: evacs of groups 0..7 + dsb) ----
    POOLZ = {3, 7, 11, 15}   # tiles 2,3 of these groups build on Pool

    def zb(eng, t0, n):      # build Z for tiles t0 .. t0+n-1
        eng.tensor_mul(
            _sl(zbuf, [(PQ, n), (26, 14), (1, 26)], t0 * PQ + 1),
            _sl(aext, [(AE, n), (0, 14), (1, 26)], t0 * AE),
            _sl(aext, [(AE, n), (1, 14), (1, 26)], t0 * AE))

    half_a = half_ops(0)
    for g in range(NG):
        if g < 2:           # small first Z ops: earlier pipeline start
            zb(nc.vector, g * 4, 4)
        elif g % 2 == 0:
            zb(nc.vector, g * 4, 6 if (g + 1) in POOLZ else 8)
        if (g + 3) in POOLZ:     # 3-group lead for the slow Pool engine
            zb(nc.gpsimd, (g + 3) * 4 + 2, 2)
        t_matmuls(g)
        evacs(g)
        if g >= 9:
            for _ in range(10):
                if half_a:
                    half_a.pop(0)()
    while half_a:
        half_a.pop(0)()
    # half-A stores overlap half-B's matmuls
    nc.scalar.copy(ress[:, 0:128], o1p[:, 0:128])
    nc.vector.tensor_scalar(ress[:, 256:384], accp[:, 0:128], b2s, None,
                            mybir.AluOpType.add)
    nc.sync.dma_start(res_d[:, 0:128], ress[:, 0:128])
    nc.sync.dma_start(res_d[:, 256:384], ress[:, 256:384])

    ops_b = half_ops(1)
    for op in ops_b[:3]:     # out1-B first; its store overlaps out2-B
        op()
    nc.scalar.copy(ress[:, 128:256], o1p[:, 128:256])
    nc.sync.dma_start(res_d[:, 128:256], ress[:, 128:256])
    for op in ops_b[3:]:
        op()
    nc.vector.tensor_scalar(ress[:, 384:512], accp[:, 128:256], b2s, None,
                            mybir.AluOpType.add)
    nc.sync.dma_start(res_d[:, 384:512], ress[:, 384:512])


def host_prep_weights(W1, b1, W2, b2):
    # C matrix [384, 128]: row 0 = b1; rows 1+d*26+p = pair coeffs.
    C = np.zeros((PQ, H), dtype=np.float32)
    C[0, :] = b1
    pair_of_row = {}
    row_of_pair = {}
    for d in range(14):
        for p in range(26):
            q = (p + d) % 26
            r = 1 + d * 26 + p
            if d == 0:
                C[r, :] = W1[:, p, p]
            elif d == 13:
                C[r, :] = 0.5 * (W1[:, p, q] + W1[:, q, p])
            else:
                C[r, :] = W1[:, p, q] + W1[:, q, p]
            key = (min(p, q), max(p, q))
            pair_of_row[r] = key
            row_of_pair.setdefault(key, r)
    csb = C.reshape(3, 128, H).transpose(1, 0, 2).reshape(128, PQ)

    # E-fold: re-bucket D cells onto canonical cells (pair of the two
    # smallest, j = max of the triple).
    Dfull = np.einsum('pi,hij->pjh', C, W2)          # [384, 26, 128]
    E = np.zeros_like(Dfull)
    E[0] = Dfull[0]
    for r in range(1, 365):
        p, q = pair_of_row[r]
        for j in range(26):
            t0, t1, t2 = sorted((p, q, j))
            E[row_of_pair[(t0, t1)], t2, :] += Dfull[r, j, :]
    blocks = []
    for c in range(3):
        for j in range(SKIP[c], 26):
            blocks.append(E[c * 128:(c + 1) * 128, j, :])
    dsb = np.concatenate(blocks, axis=1)             # [128, NJ*128]
    return (csb.astype(BF), np.ascontiguousarray(dsb).astype(BF),
            (32.0 * b2[:, None]).astype(np.float32))


def host_prep_inputs(inputs):
    """Per-core A layouts (pure relayout/padding of the input tensor)."""
    a = inputs.reshape(NC, NT, 4, 26, 32).transpose(0, 2, 4, 1, 3)
    ab = np.ascontiguousarray(a).astype(BF)      # [NC, 4, 32, NT, 26]
    ab = ab.reshape(NC, 128, NT, 26)
    aext = np.zeros((NC, 128, NT, AE), dtype=BF)
    aext[:, :, :, 0:26] = ab
    aext[:, :, :, 26:39] = ab[:, :, :, 0:13]
    asd = np.zeros((NC, 128, NT, 108), dtype=BF)
    for bl in range(4):
        asd[:, bl * 32:(bl + 1) * 32, :, bl:104 + bl:4] = \
            ab[:, bl * 32:(bl + 1) * 32]
        asd[:, bl * 32:(bl + 1) * 32, :, 104 + bl] = 1.0
    rs = lambda x: np.ascontiguousarray(x.reshape(NC, 128, -1))
    return rs(aext), rs(asd)


_nc_cache = {}


def kernel(inputs, W1, b1, W2, b2):
    inputs = np.ascontiguousarray(np.asarray(inputs, dtype=np.float32))
    W1 = np.asarray(W1, dtype=np.float32)
    b1 = np.asarray(b1, dtype=np.float32)
    W2 = np.asarray(W2, dtype=np.float32)
    b2 = np.asarray(b2, dtype=np.float32)

    csb, dsb, b2s = host_prep_weights(W1, b1, W2, b2)
    aext, asd = host_prep_inputs(inputs)

    if "nc" not in _nc_cache:
        _nc_cache["nc"] = build_nc()
    nc = _nc_cache["nc"]

    in_maps = []
    for c in range(NC):
        in_maps.append({
            "aext": aext[c], "asd": asd[c],
            "c_w": csb, "d_w": dsb, "b2s": b2s,
        })
    r = run_bass_kernel_spmd(nc, in_maps, core_ids=list(range(NC)),
                             trace=bool(int(os.environ.get("K_TRACE", "0"))))
    outs = []
    for c in range(NC):
        rc = r.results[c]["res"]                 # [128, out1(256)|out2(256)]
        outs.append(np.concatenate([rc[:, 0:256].T, rc[:, 256:512].T],
                                   axis=1))      # [256, 256]
    out = np.concatenate(outs, axis=0)
    if r.exec_time_ns is not None:
        kernel.last_exec_ns = r.exec_time_ns
    kernel.last_results = r
    return out


kernel.last_exec_ns = None
kernel.last_results = None


if __name__ == "__main__":
    import reference
    inp = {k: np.asarray(v) for k, v in reference.setup_inputs().items()}
    expected = np.asarray(reference.reference(**inp))
    got = kernel(**inp)
    err = np.abs(got - expected).max()
    rel = err / np.abs(expected).max()
    print("max abs err:", err, "rel:", rel)


# revision 21
# speedup vs baseline: 1.0728x; 1.0728x over previous
"""Trainium2 Bass kernel for the 2-layer CIN — v5.

Math (per batch b, reference):
  x1[b,h,k] = sum_{i,j} W1[h,i,j] * x[b,i,k] * x[b,j,k] + b1[h]
  x2[b,h,k] = sum_{i,j} W2[h,i,j] * x1[b,i,k] * x[b,j,k] + b2[h]
  out[b, :] = [sum_k x1[b,:,k], sum_k x2[b,:,k]]          # [B, 256]

Device strategy (pure data parallel over 8 cores, 256 batches each):
  - Columns col=(b_lo 4, k 32) on the 128 SBUF partitions; 64 col-tiles.
  - Z rows: row 0 = 1.0 (bias carrier); rows 1+d*26+p = a_p * a_{(p+d)%26}
    (d in 0..13, 0.5-coeff fold for d=13); rows 365..383 zero.
  - T[dq, (j,bl)] = sum_k Z[(bl,k), dq] * x[b_bl, j, k] via per-(tile,
    chunk) matmuls contracting over the partition dim.  Canonical-triple
    fold: T[(p,q), j] is only consumed at j >= max(p,q), so chunk 1
    skips j<5 and chunk 2 skips j<10 (exact — weights re-bucketed on
    host into E).  Per-tile moving cols: 108 / 88 / 68.
  - out2[h,b] = sum_{dq,j} E[dq,j,h] * T[dq,(j,b)] — 63 accumulating
    matmuls (26+21+16) instead of 78.
  - out1[h,b] = sum_dq C[dq,h] * ZS[dq,b] — 3 accumulating matmuls on
    the ones-columns of T.
  - Z build on DVE; PSUM evacuation split across ACT / Pool / (order)
    with 2-group fused ops.  A/B probe: even groups build Z with
    per-tile scalar_tensor_tensor (4x_2p candidate), odd groups with
    per-group tensor_tensor (2x_1p).
  - Output stays [h, b] on device; the [b, h] transpose happens on host.
"""

import dataclasses
import os
import sys

sys.path.insert(0, "/opt/trn_rl_repo")

import numpy as np
import ml_dtypes

import concourse.bass as bass
import concourse.tile as tile
from concourse import bacc
from concourse import mybir
from concourse.bass_utils import run_bass_kernel_spmd

BF = ml_dtypes.bfloat16

B, M, K, H = 2048, 26, 32, 128
NC = 8
BS = B // NC        # 256 batches per core
NT = BS // 4        # 64 col tiles
NG = NT // 4        # 16 groups of 4 tiles
PQ = 384            # packed pair dim (3 chunks of 128)
AE = 48             # per-tile stride in a_ext

SKIP = (0, 5, 10)                 # first executed j per chunk
TC = tuple((26 - s) * 4 + 4 for s in SKIP)   # T cols/tile/chunk: 108, 88, 68
NJ = sum(26 - s for s in SKIP)    # out2 matmul count: 63

F32 = mybir.dt.float32
BF16 = mybir.dt.bfloat16
GS = 432             # per-group col stride in each tsb chunk region


def _sl(ap, ap_dims, extra_off=0):
    """Raw AP with custom free dims [(step, count), ...]."""
    return dataclasses.replace(
        ap, offset=ap.offset + extra_off,
        ap=[list(ap.ap[0])] + [[s, c] for s, c in ap_dims])


def build_nc():
    nc = bacc.Bacc("TRN2", target_bir_lowering=False, debug=False,
                   num_devices=NC)

    dr = lambda n, shp, dt: nc.dram_tensor(n, shp, dt, kind="ExternalInput").ap()
    aext_d = dr("aext", [128, NT * AE], BF16)
    as_d = dr("asd", [128, NT * 108], BF16)
    c_d = dr("c_w", [128, PQ], BF16)
    d_d = dr("d_w", [128, NJ * 128], BF16)
    b2_d = dr("b2s", [128, 1], F32)
    res_d = nc.dram_tensor("res", [128, 512], F32, kind="ExternalOutput").ap()

    with tile.TileContext(nc, trace_sim=False) as tc:
        _body(nc, aext_d, as_d, c_d, d_d, b2_d, res_d)
    nc.compile()
    return nc


def _body(nc, aext_d, as_d, c_d, d_d, b2_d, res_d):
    sb = lambda n, f, dt: nc.alloc_sbuf_tensor(n, [128, f], dt).ap()

    aext = sb("aext_s", NT * AE, BF16)
    asb = sb("asb", NT * 108, BF16)
    zbuf = sb("zbuf", NT * PQ, BF16)
    # tsb: 3 chunk regions of NG*GS cols each; group g's 4 tiles live at
    # [c*NG*GS + g*GS : +4*TC[c]] (tail of each 432-block junk for c1/c2).
    tsb = sb("tsb", 3 * NG * GS, BF16)
    csb = sb("csb", PQ, BF16)
    dsb = sb("dsb", NJ * 128, BF16)
    b2s = sb("b2s_s", 1, F32)
    ress = sb("ress", 512, F32)
    wsrc = sb("wsrc", 256, BF16)        # never written: warm-up junk

    # Per-chunk PSUM pools: c0/c1 depth-2, c2 depth-3; accp+o1p share the
    # 8th bank (accp = pacc[:, :256], o1p = pacc[:, 256:]).
    pc = [nc.alloc_psum_tensor("pc0", [128, 1024], F32).ap(),
          nc.alloc_psum_tensor("pc1", [128, 1024], F32).ap(),
          nc.alloc_psum_tensor("pc2", [128, 1536], F32).ap()]
    pacc = nc.alloc_psum_tensor("pacc", [128, 512], F32).ap()
    accp = pacc[:, 0:256]
    o1p = pacc[:, 256:512]
    DEP = (2, 2, 3)

    # ---- PE warm-up: no data deps, runs from preamble end ----
    for w in range(12):
        nc.tensor.matmul(o1p, wsrc[:, 0:128], wsrc[:, 0:256],
                         start=True, stop=True, skip_group_check=True)

    # ---- loads: ONE queue = strict FIFO drain order. A chunks first
    #      (aext/asd interleaved, progressive sizes), then csb/b2, then
    #      dsb (needed only by the mid-phase out2 half-A).
    for lo, n in ((0, 4), (4, 4), (8, 8), (16, 16), (32, 32)):
        s = slice(lo * AE, (lo + n) * AE)
        nc.sync.dma_start(aext[:, s], aext_d[:, s])
        s = slice(lo * 108, (lo + n) * 108)
        nc.sync.dma_start(asb[:, s], as_d[:, s])
    nc.sync.dma_start(csb, c_d)
    nc.sync.dma_start(b2s, b2_d)
    for lo, hi in ((0, 32), (32, NJ)):
        s = slice(lo * 128, hi * 128)
        nc.sync.dma_start(dsb[:, s], d_d[:, s])

    # ---- Z bias/zero rows (once, whole zbuf) ----
    nc.gpsimd.memset(_sl(zbuf, [(PQ, NT), (1, 1)], 0), 1.0)
    nc.gpsimd.memset(_sl(zbuf, [(PQ, NT), (1, 19)], 365), 0.0)

    def t_matmuls(g):
        for c in range(3):
            po = (g % DEP[c]) * 512
            for tt in range(4):
                t = g * 4 + tt
                nc.tensor.matmul(
                    pc[c][:, po + tt * TC[c]: po + (tt + 1) * TC[c]],
                    zbuf[:, t * PQ + c * 128: t * PQ + (c + 1) * 128],
                    asb[:, t * 108 + SKIP[c] * 4:(t + 1) * 108],
                    start=True, stop=True, skip_group_check=True)

    def evacs(g):
        # exact-size per-chunk ops; c2 on DVE except the early odd groups
        # (DVE is still streaming Z then); late c1s also to DVE to
        # balance the saturated ACT.
        for c in range(3):
            dst = tsb[:, c * NG * GS + g * GS:
                         c * NG * GS + g * GS + 4 * TC[c]]
            src = pc[c][:, (g % DEP[c]) * 512:
                          (g % DEP[c]) * 512 + 4 * TC[c]]
            dve = c == 2 and not (g % 2 == 1 and g < 8)
            if dve:
                nc.vector.tensor_scalar(dst, src, 0.0, None,
                                        mybir.AluOpType.add)
            else:
                nc.scalar.copy(dst, src)

    def half_ops(h):
        """Closure list: 3 out1 then 63 out2 accumulating matmuls for
        half h (out1 first so its store can overlap the out2 chain)."""
        ops = []
        for c in range(3):
            def mm1(c=c):
                nc.tensor.matmul(
                    o1p[:, h * 128:(h + 1) * 128],
                    csb[:, c * 128:(c + 1) * 128],
                    _sl(tsb, [(GS, 8), (TC[c], 4), (1, 4)],
                        c * NG * GS + h * 8 * GS + TC[c] - 4),
                    start=(c == 0), stop=(c == 2), skip_group_check=True)
            ops.append(mm1)
        blk = 0
        for c in range(3):
            for j in range(SKIP[c], 26):
                def mm(c=c, j=j, blk=blk):
                    nc.tensor.matmul(
                        accp[:, h * 128:h * 128 + 128],
                        dsb[:, blk * 128:(blk + 1) * 128],
                        _sl(tsb, [(GS, 8), (TC[c], 4), (1, 4)],
                            c * NG * GS + h * 8 * GS + (j - SKIP[c]) * 4),
                        start=(blk == 0), stop=(blk == NJ - 1),
                        skip_group_check=True)
                ops.append(mm)
                blk += 1
        return ops

    # ---- T phase; half-A of out2/out1 issue-interleaved into the PE
    #      slack of groups 9..15 (its deps: evacs of groups 0..7 + dsb) ----
    def zb(eng, t0, n):      # build Z for tiles t0 .. t0+n-1
        eng.tensor_mul(
            _sl(zbuf, [(PQ, n), (26, 14), (1, 26)], t0 * PQ + 1),
            _sl(aext, [(AE, n), (0, 14), (1, 26)], t0 * AE),
            _sl(aext, [(AE, n), (1, 14), (1, 26)], t0 * AE))

    half_a = half_ops(0)
    for g in range(NG):
        if g < 2:           # small first Z ops: earlier pipeline start
            zb(nc.vector, g * 4, 4)
        elif g % 2 == 0:
            zb(nc.vector, g * 4, 8)
        t_matmuls(g)
        evacs(g)
        if g >= 9:
            for _ in range(10):
                if half_a:
                    half_a.pop(0)()
    while half_a:
        half_a.pop(0)()
    # half-A stores overlap half-B's matmuls
    nc.scalar.copy(ress[:, 0:128], o1p[:, 0:128])
    nc.vector.tensor_scalar(ress[:, 256:384], accp[:, 0:128], b2s, None,
                            mybir.AluOpType.add)
    nc.sync.dma_start(res_d[:, 0:128], ress[:, 0:128])
    nc.sync.dma_start(res_d[:, 256:384], ress[:, 256:384])

    ops_b = half_ops(1)
    for op in ops_b[:3]:     # out1-B first; its store overlaps out2-B
        op()
    nc.scalar.copy(ress[:, 128:256], o1p[:, 128:256])
    nc.sync.dma_start(res_d[:, 128:256], ress[:, 128:256])
    for op in ops_b[3:]:
        op()
    nc.vector.tensor_scalar(ress[:, 384:512], accp[:, 128:256], b2s, None,
                            mybir.AluOpType.add)
    nc.sync.dma_start(res_d[:, 384:512], ress[:, 384:512])


def host_prep_weights(W1, b1, W2, b2):
    # C matrix [384, 128]: row 0 = b1; rows 1+d*26+p = pair coeffs.
    C = np.zeros((PQ, H), dtype=np.float32)
    C[0, :] = b1
    pair_of_row = {}
    row_of_pair = {}
    for d in range(14):
        for p in range(26):
            q = (p + d) % 26
            r = 1 + d * 26 + p
            if d == 0:
                C[r, :] = W1[:, p, p]
            elif d == 13:
                C[r, :] = 0.5 * (W1[:, p, q] + W1[:, q, p])
            else:
                C[r, :] = W1[:, p, q] + W1[:, q, p]
            key = (min(p, q), max(p, q))
            pair_of_row[r] = key
            row_of_pair.setdefault(key, r)
    csb = C.reshape(3, 128, H).transpose(1, 0, 2).reshape(128, PQ)

    # E-fold: re-bucket D cells onto canonical cells (pair of the two
    # smallest, j = max of the triple).
    Dfull = np.einsum('pi,hij->pjh', C, W2)          # [384, 26, 128]
    E = np.zeros_like(Dfull)
    E[0] = Dfull[0]
    for r in range(1, 365):
        p, q = pair_of_row[r]
        for j in range(26):
            t0, t1, t2 = sorted((p, q, j))
            E[row_of_pair[(t0, t1)], t2, :] += Dfull[r, j, :]
    blocks = []
    for c in range(3):
        for j in range(SKIP[c], 26):
            blocks.append(E[c * 128:(c + 1) * 128, j, :])
    dsb = np.concatenate(blocks, axis=1)             # [128, NJ*128]
    return (csb.astype(BF), np.ascontiguousarray(dsb).astype(BF),
            (32.0 * b2[:, None]).astype(np.float32))


def host_prep_inputs(inputs):
    """Per-core A layouts (pure relayout/padding of the input tensor)."""
    a = inputs.reshape(NC, NT, 4, 26, 32).transpose(0, 2, 4, 1, 3)
    ab = np.ascontiguousarray(a).astype(BF)      # [NC, 4, 32, NT, 26]
    ab = ab.reshape(NC, 128, NT, 26)
    aext = np.zeros((NC, 128, NT, AE), dtype=BF)
    aext[:, :, :, 0:26] = ab
    aext[:, :, :, 26:39] = ab[:, :, :, 0:13]
    asd = np.zeros((NC, 128, NT, 108), dtype=BF)
    for bl in range(4):
        asd[:, bl * 32:(bl + 1) * 32, :, bl:104 + bl:4] = \
            ab[:, bl * 32:(bl + 1) * 32]
        asd[:, bl * 32:(bl + 1) * 32, :, 104 + bl] = 1.0
    rs = lambda x: np.ascontiguousarray(x.reshape(NC, 128, -1))
    return rs(aext), rs(asd)


_nc_cache = {}


def kernel(inputs, W1, b1, W2, b2):
    inputs = np.ascontiguousarray(np.asarray(inputs, dtype=np.float32))
    W1 = np.asarray(W1, dtype=np.float32)
    b1 = np.asarray(b1, dtype=np.float32)
    W2 = np.asarray(W2, dtype=np.float32)
    b2 = np.asarray(b2, dtype=np.float32)

    csb, dsb, b2s = host_prep_weights(W1, b1, W2, b2)
    aext, asd = host_prep_inputs(inputs)

    if "nc" not in _nc_cache:
        _nc_cache["nc"] = build_nc()
    nc = _nc_cache["nc"]

    in_maps = []
    for c in range(NC):
        in_maps.append({
            "aext": aext[c], "asd": asd[c],
            "c_w": csb, "d_w": dsb, "b2s": b2s,
        })
    r = run_bass_kernel_spmd(nc, in_maps, core_ids=list(range(NC)),
                             trace=bool(int(os.environ.get("K_TRACE", "0"))))
    outs = []
    for c in range(NC):
        rc = r.results[c]["res"]                 # [128, out1(256)|out2(256)]
        outs.append(np.concatenate([rc[:, 0:256].T, rc[:, 256:512].T],
                                   axis=1))      # [256, 256]
    out = np.concatenate(outs, axis=0)
    if r.exec_time_ns is not None:
        kernel.last_exec_ns = r.exec_time_ns
    kernel.last_results = r
    return out


kernel.last_exec_ns = None
kernel.last_results = None


if __name__ == "__main__":
    import reference
    inp = {k: np.asarray(v) for k, v in reference.setup_inputs().items()}
    expected = np.asarray(reference.reference(**inp))
    got = kernel(**inp)
    err = np.abs(got - expected).max()
    rel = err / np.abs(expected).max()
    print("max abs err:", err, "rel:", rel)


# revision 24
# speedup vs baseline: 1.1356x; 1.0586x over previous
"""Trainium2 Bass kernel for the 2-layer CIN — v5.

Math (per batch b, reference):
  x1[b,h,k] = sum_{i,j} W1[h,i,j] * x[b,i,k] * x[b,j,k] + b1[h]
  x2[b,h,k] = sum_{i,j} W2[h,i,j] * x1[b,i,k] * x[b,j,k] + b2[h]
  out[b, :] = [sum_k x1[b,:,k], sum_k x2[b,:,k]]          # [B, 256]

Device strategy (pure data parallel over 8 cores, 256 batches each):
  - Columns col=(b_lo 4, k 32) on the 128 SBUF partitions; 64 col-tiles.
  - Z rows: row 0 = 1.0 (bias carrier); rows 1+d*26+p = a_p * a_{(p+d)%26}
    (d in 0..13, 0.5-coeff fold for d=13); rows 365..383 zero.
  - T[dq, (j,bl)] = sum_k Z[(bl,k), dq] * x[b_bl, j, k] via per-(tile,
    chunk) matmuls contracting over the partition dim.  Canonical-triple
    fold: T[(p,q), j] is only consumed at j >= max(p,q), so chunk 1
    skips j<5 and chunk 2 skips j<10 (exact — weights re-bucketed on
    host into E).  Per-tile moving cols: 108 / 88 / 68.
  - out2[h,b] = sum_{dq,j} E[dq,j,h] * T[dq,(j,b)] — 63 accumulating
    matmuls (26+21+16) instead of 78.
  - out1[h,b] = sum_dq C[dq,h] * ZS[dq,b] — 3 accumulating matmuls on
    the ones-columns of T.
  - Z build on DVE; PSUM evacuation split across ACT / Pool / (order)
    with 2-group fused ops.  A/B probe: even groups build Z with
    per-tile scalar_tensor_tensor (4x_2p candidate), odd groups with
    per-group tensor_tensor (2x_1p).
  - Output stays [h, b] on device; the [b, h] transpose happens on host.
"""

import dataclasses
import os
import sys

sys.path.insert(0, "/opt/trn_rl_repo")

import numpy as np
import ml_dtypes

import concourse.bass as bass
import concourse.tile as tile
from concourse import bacc
from concourse import mybir
from concourse.bass_utils import run_bass_kernel_spmd

BF = ml_dtypes.bfloat16

B, M, K, H = 2048, 26, 32, 128
NC = 8
BS = B // NC        # 256 batches per core
NT = BS // 4        # 64 col tiles
NG = NT // 4        # 16 groups of 4 tiles
PQ = 384            # packed pair dim (3 chunks of 128)
AE = 48             # per-tile stride in a_ext

SKIP = (0, 5, 10)                 # first executed j per chunk
TC = tuple((26 - s) * 4 + 4 for s in SKIP)   # T cols/tile/chunk: 108, 88, 68
NJ = sum(26 - s for s in SKIP)    # out2 matmul count: 63

F32 = mybir.dt.float32
BF16 = mybir.dt.bfloat16
GS = 432             # per-group col stride in each tsb chunk region


def _sl(ap, ap_dims, extra_off=0):
    """Raw AP with custom free dims [(step, count), ...]."""
    return dataclasses.replace(
        ap, offset=ap.offset + extra_off,
        ap=[list(ap.ap[0])] + [[s, c] for s, c in ap_dims])


def build_nc():
    nc = bacc.Bacc("TRN2", target_bir_lowering=False, debug=False,
                   num_devices=NC)

    dr = lambda n, shp, dt: nc.dram_tensor(n, shp, dt, kind="ExternalInput").ap()
    aext_d = dr("aext", [128, NT * AE], BF16)
    as_d = dr("asd", [128, NT * 108], BF16)
    c_d = dr("c_w", [128, PQ], BF16)
    d_d = dr("d_w", [128, NJ * 128], BF16)
    b2_d = dr("b2s", [128, 1], F32)
    res_d = nc.dram_tensor("res", [128, 512], F32, kind="ExternalOutput").ap()

    with tile.TileContext(nc, trace_sim=False) as tc:
        _body(nc, aext_d, as_d, c_d, d_d, b2_d, res_d)
    nc.compile()
    return nc


def _body(nc, aext_d, as_d, c_d, d_d, b2_d, res_d):
    sb = lambda n, f, dt: nc.alloc_sbuf_tensor(n, [128, f], dt).ap()

    aext = sb("aext_s", NT * AE, BF16)
    asb = sb("asb", NT * 108, BF16)
    zbuf = sb("zbuf", NT * PQ, BF16)
    # tsb: 3 chunk regions of NG*GS cols each; group g's 4 tiles live at
    # [c*NG*GS + g*GS : +4*TC[c]] (tail of each 432-block junk for c1/c2).
    tsb = sb("tsb", 3 * NG * GS, BF16)
    csb = sb("csb", PQ, BF16)
    dsb = sb("dsb", NJ * 128, BF16)
    b2s = sb("b2s_s", 1, F32)
    ress = sb("ress", 512, F32)
    wsrc = sb("wsrc", 256, BF16)        # never written: warm-up junk

    # Per-chunk PSUM pools: c0/c1 depth-2, c2 depth-3; accp+o1p share the
    # 8th bank (accp = pacc[:, :256], o1p = pacc[:, 256:]).
    pc = [nc.alloc_psum_tensor("pc0", [128, 1024], F32).ap(),
          nc.alloc_psum_tensor("pc1", [128, 1024], F32).ap(),
          nc.alloc_psum_tensor("pc2", [128, 1536], F32).ap()]
    pacc = nc.alloc_psum_tensor("pacc", [128, 512], F32).ap()
    accp = pacc[:, 0:256]
    o1p = pacc[:, 256:512]
    DEP = (2, 2, 3)

    # ---- PE warm-up: no data deps, runs from preamble end ----
    for w in range(14):
        nc.tensor.matmul(o1p, wsrc[:, 0:128], wsrc[:, 0:256],
                         start=True, stop=True, skip_group_check=True)

    # ---- loads: ONE queue = strict FIFO drain order. A chunks first
    #      (aext/asd interleaved, progressive sizes), then csb/b2, then
    #      dsb (needed only by the mid-phase out2 half-A).
    for lo, n in ((0, 4), (4, 4), (8, 8), (16, 16), (32, 32)):
        s = slice(lo * AE, (lo + n) * AE)
        nc.sync.dma_start(aext[:, s], aext_d[:, s])
        s = slice(lo * 108, (lo + n) * 108)
        nc.sync.dma_start(asb[:, s], as_d[:, s])
    nc.sync.dma_start(csb, c_d)
    nc.sync.dma_start(b2s, b2_d)
    for lo, hi in ((0, 32), (32, NJ)):
        s = slice(lo * 128, hi * 128)
        nc.sync.dma_start(dsb[:, s], d_d[:, s])

    # ---- Z bias/zero rows (once, whole zbuf) ----
    nc.gpsimd.memset(_sl(zbuf, [(PQ, NT), (1, 1)], 0), 1.0)
    nc.gpsimd.memset(_sl(zbuf, [(PQ, NT), (1, 19)], 365), 0.0)

    def t_matmuls(g):
        for c in range(3):
            po = (g % DEP[c]) * 512
            for tt in range(4):
                t = g * 4 + tt
                nc.tensor.matmul(
                    pc[c][:, po + tt * TC[c]: po + (tt + 1) * TC[c]],
                    zbuf[:, t * PQ + c * 128: t * PQ + (c + 1) * 128],
                    asb[:, t * 108 + SKIP[c] * 4:(t + 1) * 108],
                    start=True, stop=True, skip_group_check=True)

    def evacs(g):
        # exact-size per-chunk ops; c2 on DVE except the early odd groups
        # (DVE is still streaming Z then); late c1s also to DVE to
        # balance the saturated ACT.
        for c in range(3):
            dst = tsb[:, c * NG * GS + g * GS:
                         c * NG * GS + g * GS + 4 * TC[c]]
            src = pc[c][:, (g % DEP[c]) * 512:
                          (g % DEP[c]) * 512 + 4 * TC[c]]
            dve = c == 2 and not (g % 2 == 1 and g < 8)
            if dve:
                nc.vector.tensor_scalar(dst, src, 0.0, None,
                                        mybir.AluOpType.add)
            else:
                nc.scalar.copy(dst, src)

    def half_ops(h):
        """Closure list: 63 out2 + 3 out1 accumulating matmuls for half h."""
        ops = []
        blk = 0
        for c in range(3):
            for j in range(SKIP[c], 26):
                def mm(c=c, j=j, blk=blk):
                    nc.tensor.matmul(
                        accp[:, h * 128:h * 128 + 128],
                        dsb[:, blk * 128:(blk + 1) * 128],
                        _sl(tsb, [(GS, 8), (TC[c], 4), (1, 4)],
                            c * NG * GS + h * 8 * GS + (j - SKIP[c]) * 4),
                        start=(blk == 0), stop=(blk == NJ - 1),
                        skip_group_check=True)
                ops.append(mm)
                blk += 1
        for c in range(3):
            def mm1(c=c):
                nc.tensor.matmul(
                    o1p[:, h * 128:(h + 1) * 128],
                    csb[:, c * 128:(c + 1) * 128],
                    _sl(tsb, [(GS, 8), (TC[c], 4), (1, 4)],
                        c * NG * GS + h * 8 * GS + TC[c] - 4),
                    start=(c == 0), stop=(c == 2), skip_group_check=True)
            ops.append(mm1)
        return ops

    # ---- T phase; half-A of out2/out1 issue-interleaved into the PE
    #      slack of groups 9..15 (its deps: evacs of groups 0..7 + dsb) ----
    def zb(eng, t0, n):      # build Z for tiles t0 .. t0+n-1
        eng.tensor_mul(
            _sl(zbuf, [(PQ, n), (26, 14), (1, 26)], t0 * PQ + 1),
            _sl(aext, [(AE, n), (0, 14), (1, 26)], t0 * AE),
            _sl(aext, [(AE, n), (1, 14), (1, 26)], t0 * AE))

    half_a = half_ops(0)
    for g in range(NG):
        if g < 2:           # small first Z ops: earlier pipeline start
            zb(nc.vector, g * 4, 4)
        elif g % 2 == 0:
            zb(nc.vector, g * 4, 8)
        t_matmuls(g)
        evacs(g)
        if g >= 9:
            for _ in range(10):
                if half_a:
                    half_a.pop(0)()
    while half_a:
        half_a.pop(0)()
    # half-A stores overlap half-B's matmuls
    nc.scalar.copy(ress[:, 0:128], o1p[:, 0:128])
    nc.vector.tensor_scalar(ress[:, 256:384], accp[:, 0:128], b2s, None,
                            mybir.AluOpType.add)
    nc.sync.dma_start(res_d[:, 0:128], ress[:, 0:128])
    nc.sync.dma_start(res_d[:, 256:384], ress[:, 256:384])

    for op in half_ops(1):
        op()
    nc.scalar.copy(ress[:, 128:256], o1p[:, 128:256])
    nc.vector.tensor_scalar(ress[:, 384:512], accp[:, 128:256], b2s, None,
                            mybir.AluOpType.add)
    nc.sync.dma_start(res_d[:, 128:256], ress[:, 128:256])
    nc.sync.dma_start(res_d[:, 384:512], ress[:, 384:512])


def host_prep_weights(W1, b1, W2, b2):
    # C matrix [384, 128]: row 0 = b1; rows 1+d*26+p = pair coeffs.
    C = np.zeros((PQ, H), dtype=np.float32)
    C[0, :] = b1
    pair_of_row = {}
    row_of_pair = {}
    for d in range(14):
        for p in range(26):
            q = (p + d) % 26
            r = 1 + d * 26 + p
            if d == 0:
                C[r, :] = W1[:, p, p]
            elif d == 13:
                C[r, :] = 0.5 * (W1[:, p, q] + W1[:, q, p])
            else:
                C[r, :] = W1[:, p, q] + W1[:, q, p]
            key = (min(p, q), max(p, q))
            pair_of_row[r] = key
            row_of_pair.setdefault(key, r)
    csb = C.reshape(3, 128, H).transpose(1, 0, 2).reshape(128, PQ)

    # E-fold: re-bucket D cells onto canonical cells (pair of the two
    # smallest, j = max of the triple).
    Dfull = np.einsum('pi,hij->pjh', C, W2)          # [384, 26, 128]
    E = np.zeros_like(Dfull)
    E[0] = Dfull[0]
    for r in range(1, 365):
        p, q = pair_of_row[r]
        for j in range(26):
            t0, t1, t2 = sorted((p, q, j))
            E[row_of_pair[(t0, t1)], t2, :] += Dfull[r, j, :]
    blocks = []
    for c in range(3):
        for j in range(SKIP[c], 26):
            blocks.append(E[c * 128:(c + 1) * 128, j, :])
    dsb = np.concatenate(blocks, axis=1)             # [128, NJ*128]
    return (csb.astype(BF), np.ascontiguousarray(dsb).astype(BF),
            (32.0 * b2[:, None]).astype(np.float32))


def host_prep_inputs(inputs):
    """Per-core A layouts (pure relayout/padding of the input tensor)."""
    a = inputs.reshape(NC, NT, 4, 26, 32).transpose(0, 2, 4, 1, 3)
    ab = np.ascontiguousarray(a).astype(BF)      # [NC, 4, 32, NT, 26]
    ab = ab.reshape(NC, 128, NT, 26)
    aext = np.zeros((NC, 128, NT, AE), dtype=BF)
    aext[:, :, :, 0:26] = ab
    aext[:, :, :, 26:39] = ab[:, :, :, 0:13]
    asd = np.zeros((NC, 128, NT, 108), dtype=BF)
    for bl in range(4):
        asd[:, bl * 32:(bl + 1) * 32, :, bl:104 + bl:4] = \
            ab[:, bl * 32:(bl + 1) * 32]
        asd[:, bl * 32:(bl + 1) * 32, :, 104 + bl] = 1.0
    rs = lambda x: np.ascontiguousarray(x.reshape(NC, 128, -1))
    return rs(aext), rs(asd)


_nc_cache = {}


def kernel(inputs, W1, b1, W2, b2):
    inputs = np.ascontiguousarray(np.asarray(inputs, dtype=np.float32))
    W1 = np.asarray(W1, dtype=np.float32)
    b1 = np.asarray(b1, dtype=np.float32)
    W2 = np.asarray(W2, dtype=np.float32)
    b2 = np.asarray(b2, dtype=np.float32)

    csb, dsb, b2s = host_prep_weights(W1, b1, W2, b2)
    aext, asd = host_prep_inputs(inputs)

    if "nc" not in _nc_cache:
        _nc_cache["nc"] = build_nc()
    nc = _nc_cache["nc"]

    in_maps = []
    for c in range(NC):
        in_maps.append({
            "aext": aext[c], "asd": asd[c],
            "c_w": csb, "d_w": dsb, "b2s": b2s,
        })
    r = run_bass_kernel_spmd(nc, in_maps, core_ids=list(range(NC)),
                             trace=bool(int(os.environ.get("K_TRACE", "0"))))
    outs = []
    for c in range(NC):
        rc = r.results[c]["res"]                 # [128, out1(256)|out2(256)]
        outs.append(np.concatenate([rc[:, 0:256].T, rc[:, 256:512].T],
                                   axis=1))      # [256, 256]
    out = np.concatenate(outs, axis=0)
    if r.exec_time_ns is not None:
        kernel.last_exec_ns = r.exec_time_ns
    kernel.last_results = r
    return out


kernel.last_exec_ns = None
kernel.last_results = None


if __name__ == "__main__":
    import reference
    inp = {k: np.asarray(v) for k, v in reference.setup_inputs().items()}
    expected = np.asarray(reference.reference(**inp))
    got = kernel(**inp)
    err = np.abs(got - expected).max()
    rel = err / np.abs(expected).max()
    print("max abs err:", err, "rel:", rel)
